# revision 1
# baseline (speedup 1.0000x reference)
"""CARAFE kernel for Trainium2 (8 NeuronCores, batch-parallel).

Reference computation per image:
  R = relu(conv1x1(x, w_compress, b_compress))          [48, 128, 128]
  E = conv3x3(R, w_encoder, b_encoder, pad=1)           [100, 128, 128]
  Y = softmax over k of E.reshape(4, 25, H, W)          (s, k, h, w)
  out[s,c,h,w] = sum_k Y[s,k,h,w] * xpad[c, h+dy, w+dx] (k=(dy,dx), 5x5, pad 2)
  pixel-shuffle: out_ref[s*16 + c//4, 2h + (c//2)%2, 2w + c%2] = out[s,c,h,w]

Mapping:
  - conv1x1 / conv3x3 / softmax-denominator: TensorE matmuls (channel-major),
    exp on ScalarE.  Biases folded in via a constant ones row (K=65 / K=49).
  - softmax normalization folded into the F-transpose epilogue on ScalarE.
  - The per-pixel weighted patch sum runs on VectorE in pixel-major layout
    [128 h-partitions, (c, w) free]: one mult + one add per (s, k) tap with
    the per-pixel weight broadcast along c via a free-dim step-0 AP.
    dy taps select one of five partition-shifted copies of X^T (built by
    DMA; compute engines cannot shift partitions), dx taps are free-dim
    offsets into a w-padded buffer (zero padding gives the conv edge
    semantics for free).
"""

import sys

import numpy as np

sys.path.insert(0, "/opt/trn_rl_repo")

import concourse.bass as bass
import concourse.mybir as mybir
import concourse.tile as tile
from concourse import bacc
from concourse.masks import make_identity

F32 = mybir.dt.float32

H = 128
W = 128
C = 64
M = 48  # compressed channels
S2 = 4  # scale_factor**2
K2 = 25  # k_up**2
SK = 100
HW = H * W
WPAD = W + 4  # w-padded pixel-major buffers
N_CORES = 8


def _ap(t, extra_off, dims):
    """Raw AP on a tile handle `t` with free-offset `extra_off` (elements)
    and explicit [step, count] dims (dims[0] is the partition dim)."""
    base = t[:]
    return bass.AP(tensor=base.tensor, offset=base.offset + extra_off, ap=dims)


class _Pool:
    """Manually scoped tile pool."""

    def __init__(self, tc, **kw):
        self._cm = tc.tile_pool(**kw)
        self.pool = self._cm.__enter__()
        self._n = 0

    def tile(self, *a, tag=None, **kw):
        self._n += 1
        t = tag or f"t{self._n}"
        return self.pool.tile(*a, tag=t, name=t, **kw)

    def close(self):
        self._cm.__exit__(None, None, None)


def build_program(debug=False, reps=1):
    nc = bacc.Bacc("TRN2", target_bir_lowering=False, debug=False)

    xin = nc.dram_tensor("xin", [C, HW], F32, kind="ExternalInput")
    w1t = nc.dram_tensor("w1t", [C + 1, M], F32, kind="ExternalInput")
    wet = nc.dram_tensor("wet", [M + 1, 9 * SK], F32, kind="ExternalInput")
    sones = nc.dram_tensor("sones", [SK, S2], F32, kind="ExternalInput")
    onesr = nc.dram_tensor("onesr", [1, 130 * 130], F32, kind="ExternalInput")
    zer = nc.dram_tensor("zer", [2, C * WPAD], F32, kind="ExternalInput")
    out = nc.dram_tensor("out", [C, 2 * H, 2 * W], F32, kind="ExternalOutput")
    dbg = {}
    if debug:
        dbg["R"] = nc.dram_tensor("dbgR", [M, HW], F32, kind="ExternalOutput")
        dbg["F"] = nc.dram_tensor("dbgF", [SK, HW], F32, kind="ExternalOutput")
        dbg["FR"] = nc.dram_tensor("dbgFR", [128, SK * W], F32, kind="ExternalOutput")
        dbg["XT"] = nc.dram_tensor("dbgXT", [128, C * WPAD], F32, kind="ExternalOutput")

    with tile.TileContext(nc) as tc:
        cp = _Pool(tc, name="consts", bufs=1)
        pp = _Pool(tc, name="persist", bufs=1)

        w1t_sb = cp.tile([C + 1, M], F32)
        nc.sync.dma_start(w1t_sb[:], w1t.ap())
        wet_sb = cp.tile([M + 1, 9 * SK], F32)
        nc.sync.dma_start(wet_sb[:], wet.ap())
        sones_sb = cp.tile([SK, S2], F32)
        nc.sync.dma_start(sones_sb[:], sones.ap())
        ident = cp.tile([128, 128], F32)
        make_identity(nc, ident[:])
        rzbuf = pp.tile([128, S2 * W], F32)
        xt_base = pp.tile([128, C * WPAD], F32)

        for _rep in range(reps):
            # ---- load x (+ ones row) ----
            px = _Pool(tc, name="px", bufs=1)
            x_aug = px.tile([C + 1, HW], F32)
            nc.sync.dma_start(x_aug[0:C, :], xin.ap())
            nc.sync.dma_start(
                _ap(x_aug, C * HW, [[HW, 1], [1, HW]]), onesr.ap()[:, 0:HW]
            )

            # ---- X^T via PE transpose -> XT_base [128(h), (c, WPAD)] ----
            nc.vector.memset(xt_base[:], 0.0)
            psX = _Pool(tc, name="psX", bufs=2, space="PSUM")
            for w in range(W):
                psx = psX.tile([128, C], F32, tag="psx")
                nc.tensor.transpose(
                    psx[:], _ap(x_aug, w, [[HW, C], [W, H]]), ident[0:C, 0:C]
                )
                nc.scalar.copy(
                    _ap(xt_base, 2 + w, [[C * WPAD, 128], [WPAD, C]]), psx[:]
                )
            psX.close()
            if debug:
                nc.sync.dma_start(dbg["XT"].ap(), xt_base[:])

            # ---- pass 1 (banded): conv1x1 -> relu -> r_band; conv3x3 -> exp -> f_dram; Z ----
            f_dram = nc.dram_tensor(f"fstage{_rep}", [SK, HW], F32, kind="Internal")
            BH = 32  # band height
            RB = BH + 2  # rows held per band (1-halo each side)
            RBF = RB * 130
            pband = _Pool(tc, name="pband", bufs=2)
            psA = _Pool(tc, name="psA", bufs=2, space="PSUM")
            psB = _Pool(tc, name="psB", bufs=2, space="PSUM")
            psBsb = _Pool(tc, name="psBsb", bufs=2)

            def conv1x1_rows(r_band, h0, nrows, loc0):
                """conv1x1+relu for image rows [h0, h0+nrows) into band-local row loc0."""
                ps1 = psA.tile([M, 512], F32, tag="ps1")
                nc.tensor.matmul(
                    ps1[:, 0 : nrows * W],
                    w1t_sb[:],
                    x_aug[:, h0 * W : (h0 + nrows) * W],
                    start=True,
                    stop=True,
                )
                nc.scalar.activation(
                    _ap(r_band, loc0 * 130 + 1, [[RBF, M], [130, nrows], [1, W]]),
                    ps1[:, 0 : nrows * W],
                    mybir.ActivationFunctionType.Relu,
                )

            for b in range(4):
                r_band = pband.tile([M + 1, RBF], F32, tag="rband")
                nc.gpsimd.memset(r_band[:], 0.0)
                nc.sync.dma_start(
                    _ap(r_band, M * RBF, [[RBF, 1], [1, RBF]]), onesr.ap()[:, 0:RBF]
                )
                # band covers image rows 32b-1 .. 32b+32 at band-local rows 0..33
                if b > 0:
                    conv1x1_rows(r_band, 32 * b - 1, 1, 0)
                for j in range(8):
                    conv1x1_rows(r_band, 32 * b + 4 * j, 4, 1 + 4 * j)
                if b < 3:
                    conv1x1_rows(r_band, 32 * b + 32, 1, 33)
                for j in range(8):
                    ps2 = psB.tile([SK, 512], F32, tag="ps2")
                    for t in range(9):
                        ty, tx = divmod(t, 3)
                        nc.tensor.matmul(
                            ps2[:],
                            wet_sb[:, t * SK : (t + 1) * SK],
                            _ap(
                                r_band,
                                (4 * j + ty) * 130 + tx,
                                [[RBF, M + 1], [130, 4], [1, W]],
                            ),
                            start=(t == 0),
                            stop=(t == 8),
                        )
                    fc = psBsb.tile([SK, 512], F32, tag="fc")
                    nc.scalar.activation(
                        fc[:], ps2[:], mybir.ActivationFunctionType.Exp
                    )
                    n = 8 * b + j
                    nc.sync.dma_start(
                        f_dram.ap()[:, n * 512 : (n + 1) * 512], fc[:]
                    )
                    psz = psB.tile([S2, 512], F32, tag="psz")
                    nc.tensor.matmul(
                        psz[:], sones_sb[:], fc[:], start=True, stop=True
                    )
                    zc = psBsb.tile([S2, 512], F32, tag="zc")
                    nc.scalar.copy(zc[:], psz[:])
                    # scatter Z into rzbuf [128(h), (s, w)]: rows 4n..4n+3
                    for s in range(S2):
                        nc.sync.dma_start(
                            _ap(
                                rzbuf,
                                4 * n * (S2 * W) + s * W,
                                [[S2 * W, 4], [1, W]],
                            ),
                            _ap(zc, s * 512, [[512, 1], [W, 4], [1, W]]),
                        )
            psBsb.close()
            psB.close()
            psA.close()
            pband.close()
            px.close()

            nc.vector.reciprocal(rzbuf[:], rzbuf[:])

            # ---- pass 2: reload F, transposes ----
            pfr = _Pool(tc, name="pfr", bufs=1)
            fr = pfr.tile([128, SK * W], F32)
            pf = _Pool(tc, name="pf", bufs=1)
            f_sb = pf.tile([SK, HW], F32)
            nc.sync.dma_start(f_sb[:], f_dram.ap())
            if debug:
                nc.sync.dma_start(dbg["F"].ap(), f_sb[:])


            # ---- F^T transposes + softmax-normalize -> FR [128(h), (sk, w)] ----
            psF = _Pool(tc, name="psF", bufs=2, space="PSUM")
            for w in range(W):
                pst = psF.tile([128, SK], F32, tag="pst")
                nc.tensor.transpose(
                    pst[:], _ap(f_sb, w, [[HW, SK], [W, H]]), ident[0:SK, 0:SK]
                )
                for s in range(S2):
                    nc.scalar.activation(
                        _ap(fr, (s * K2) * W + w, [[SK * W, 128], [W, K2]]),
                        pst[:, s * K2 : (s + 1) * K2],
                        mybir.ActivationFunctionType.Copy,
                        scale=rzbuf[:, s * W + w : s * W + w + 1],
                    )
            psF.close()
            pf.close()
            if debug:
                nc.sync.dma_start(dbg["FR"].ap(), fr[:])

            # ---- per-pixel patch sum on VectorE ----
            WHF = W // 2  # 64 output w per half
            XF = C * (WHF + 4)
            xtp = _Pool(tc, name="xtd", bufs=2)
            accp = _Pool(tc, name="acc", bufs=1)
            tmpp = _Pool(tc, name="tmp", bufs=1)
            acc2p = _Pool(tc, name="acc2", bufs=1)
            for half in range(2):
                for s in range(S2):
                    acc = accp.tile([128, C * WHF], F32, tag="acc")
                    for dy in range(-2, 3):
                        xtd = xtp.tile([128, XF], F32, tag="xtd")
                        p0, p1 = max(0, -dy), 128 - max(0, dy)
                        # body: partition-shifted, w-windowed copy of XT_base
                        nc.sync.dma_start(
                            _ap(xtd, p0 * XF, [[XF, p1 - p0], [1, XF]]),
                            _ap(
                                xt_base,
                                (p0 + dy) * (C * WPAD) + half * WHF,
                                [[C * WPAD, p1 - p0], [WPAD, C], [1, WHF + 4]],
                            ),
                        )
                        if p0 > 0:  # top halo rows <- zeros
                            nc.sync.dma_start(
                                _ap(xtd, 0, [[XF, p0], [1, XF]]), zer.ap()[0:p0, 0:XF]
                            )
                        if p1 < 128:  # bottom halo rows <- zeros
                            nc.sync.dma_start(
                                _ap(xtd, p1 * XF, [[XF, 128 - p1], [1, XF]]),
                                zer.ap()[0 : 128 - p1, 0:XF],
                            )
                        for dx in range(-2, 3):
                            k = (dy + 2) * 5 + (dx + 2)
                            sk = s * K2 + k
                            in0 = _ap(
                                xtd, 2 + dx, [[XF, 128], [WHF + 4, C], [1, WHF]]
                            )
                            in1 = _ap(
                                fr,
                                sk * W + half * WHF,
                                [[SK * W, 128], [0, C], [1, WHF]],
                            )
                            dst3 = _ap(acc, 0, [[C * WHF, 128], [WHF, C], [1, WHF]])
                            if k == 0:
                                nc.vector.tensor_mul(dst3, in0, in1)
                            else:
                                tmp = tmpp.tile([128, C * WHF], F32, tag="tmp")
                                t3 = _ap(tmp, 0, [[C * WHF, 128], [WHF, C], [1, WHF]])
                                nc.vector.tensor_mul(t3, in0, in1)
                                nc.vector.tensor_add(acc[:], acc[:], tmp[:])
                    # reshuffle (c, w) -> (c4, c2, w, c1) and DMA out
                    acc2 = acc2p.tile([128, C * WHF], F32, tag="acc2")
                    nc.scalar.copy(
                        acc2[:].rearrange(
                            "p (a b w d) -> p a b w d", a=16, b=2, w=WHF
                        ),
                        _ap(
                            acc,
                            0,
                            [
                                [C * WHF, 128],
                                [4 * WHF, 16],
                                [2 * WHF, 2],
                                [1, WHF],
                                [WHF, 2],
                            ],
                        ),
                    )
                    # out[s*16+c4, 2h+c2, 2*(half*64+w)+c1]; split per c2
                    for c2 in range(2):
                        dst = bass.AP(
                            tensor=out,
                            offset=(s * 16) * (4 * HW) + c2 * (2 * W) + half * W,
                            ap=[
                                [2 * (2 * W), 128],  # h -> row 2h
                                [4 * HW, 16],  # c4
                                [1, 2 * WHF],  # (w, c1) contiguous
                            ],
                        )
                        src = _ap(
                            acc2,
                            c2 * (2 * WHF),
                            [[C * WHF, 128], [4 * WHF, 16], [1, 2 * WHF]],
                        )
                        nc.sync.dma_start(dst, src)
            acc2p.close()
            tmpp.close()
            accp.close()
            xtp.close()
            pfr.close()
        pp.close()
        cp.close()
    nc.compile()
    return nc, dbg


def host_inputs(x_img, w_compress, b_compress, w_encoder, b_encoder):
    """Per-core input map for one image [C, H, W]."""
    w1t = np.concatenate(
        [w_compress[:, :, 0, 0].T, b_compress[None, :]], axis=0
    ).astype(np.float32)
    wet = np.zeros((M + 1, 9, SK), np.float32)
    for ty in range(3):
        for tx in range(3):
            wet[:M, ty * 3 + tx, :] = w_encoder[:, :, ty, tx].T
    wet[M, 4, :] = b_encoder
    son = np.zeros((SK, S2), np.float32)
    for s in range(S2):
        son[s * K2 : (s + 1) * K2, s] = 1.0
    return {
        "xin": np.ascontiguousarray(x_img.reshape(C, HW)).astype(np.float32),
        "w1t": w1t,
        "wet": wet.reshape(M + 1, 9 * SK),
        "sones": son,
        "onesr": np.ones((1, 130 * 130), np.float32),
        "zer": np.zeros((2, C * WPAD), np.float32),
    }


_CACHE = {}


def kernel(x, w_compress, b_compress, w_encoder, b_encoder):
    x = np.asarray(x, np.float32)
    if "nc" not in _CACHE:
        _CACHE["nc"], _ = build_program(debug=False)
    nc = _CACHE["nc"]
    in_maps = [
        host_inputs(
            x[i],
            np.asarray(w_compress, np.float32),
            np.asarray(b_compress, np.float32),
            np.asarray(w_encoder, np.float32),
            np.asarray(b_encoder, np.float32),
        )
        for i in range(N_CORES)
    ]
    from concourse.bass_utils import run_bass_kernel_spmd

    res = run_bass_kernel_spmd(nc, in_maps, core_ids=list(range(N_CORES)))
    return np.stack([res.results[i]["out"] for i in range(N_CORES)], axis=0)



# revision 10
# speedup vs baseline: 3.1321x; 3.1321x over previous
"""CARAFE kernel for Trainium2 (8 NeuronCores, batch-parallel), bf16 pipeline.

Reference computation per image:
  R = relu(conv1x1(x, w_compress, b_compress))          [48, 128, 128]
  E = conv3x3(R, w_encoder, b_encoder, pad=1)           [100, 128, 128]
  Y = softmax over k of E.reshape(4, 25, H, W)          (s, k, h, w)
  out[s,c,h,w] = sum_k Y[s,k,h,w] * xpad[c, h+dy, w+dx] (k=(dy,dx), 5x5, pad 2)
  pixel-shuffle: out_ref[s*16 + c//4, 2h + (c//2)%2, 2w + c%2] = out[s,c,h,w]

Mapping (single pass, SBUF-resident, bf16 data / fp32 PSUM):
  - conv1x1 + conv3x3: TensorE matmuls (channel-major), biases via ones rows.
    R kept zero-padded [49, 130*130] so conv taps are free-dim offsets.
  - exp on ScalarE (PSUM -> SBUF bf16), F^T via 128 PE transposes written
    (value-duplicated) into pixel-major fr2 [h, (s,k,w,2)].
  - softmax denominator: DVE strided reduce over k; reciprocal on DVE;
    normalization applied in-place on GpSimd per s-group (fr2 *= recipZ).
    The duplicated innermost pair keeps every apply operand 4B-aligned so
    the DVE auto-selects its 2x bf16 mode.
  - apply on DVE in pixel-major [h, (c_hi, w, c_lo)] channel-pair layout:
    per (s, w-half): 25 muls + 19 in-group adds (bf16, 2x) building 5
    dy-partials, merged into an fp32 accumulator (5 ops at 1x) to keep the
    25-term sum accurate; dy taps select one of 5 partition-shifted
    X copies (host-prepared HBM rows), dx taps are free-dim offsets.
  - pixel-shuffle falls out of the channel-pair layout: output DMA writes
    (w, c_lo) runs of 256 contiguous bf16 per (c4, row).
"""

import sys

import numpy as np

sys.path.insert(0, "/opt/trn_rl_repo")

import ml_dtypes

import concourse.bass as bass
import concourse.mybir as mybir
import concourse.tile as tile
from concourse import bacc
from concourse.masks import make_identity

F32 = mybir.dt.float32
BF16 = mybir.dt.bfloat16
BF_NP = ml_dtypes.bfloat16

H = 128
W = 128
C = 64
CH = 32  # channel pairs
M = 48  # compressed channels
S2 = 4  # scale_factor**2
K2 = 25  # k_up**2
SK = 100
HW = H * W
RP = 130  # padded R row pitch
WQ = 132  # padded w for the interleaved X^T buffer
XF = CH * WQ * 2  # 8448 free elements of each xtd tile
F2 = SK * W * 2  # 25600 free elements of fr2
N_CORES = 8

MULT = mybir.AluOpType.mult


def _ap(t, extra_off, dims):
    """Raw AP on a tile handle `t` with free-offset `extra_off` (elements)
    and explicit [step, count] dims (dims[0] is the partition dim)."""
    base = t[:]
    return bass.AP(tensor=base.tensor, offset=base.offset + extra_off, ap=dims)


class _Pool:
    """Manually scoped tile pool."""

    def __init__(self, tc, **kw):
        self._cm = tc.tile_pool(**kw)
        self.pool = self._cm.__enter__()
        self._n = 0

    def tile(self, *a, tag=None, **kw):
        self._n += 1
        t = tag or f"t{self._n}"
        return self.pool.tile(*a, tag=t, name=t, **kw)

    def close(self):
        self._cm.__exit__(None, None, None)


def build_program(debug=False):
    nc = bacc.Bacc("TRN2", target_bir_lowering=False, debug=False)

    x_aug = nc.dram_tensor("x_aug", [C + 1, HW], BF16, kind="ExternalInput")
    w1t = nc.dram_tensor("w1t", [C + 1, M], BF16, kind="ExternalInput")
    wet = nc.dram_tensor("wet", [M + 1, 9 * SK], BF16, kind="ExternalInput")
    xtq = nc.dram_tensor("xtq", [WQ, XF], BF16, kind="ExternalInput")
    onesr = nc.dram_tensor("onesr", [1, RP * RP], BF16, kind="ExternalInput")
    out = nc.dram_tensor("out", [C, 2 * H, 2 * W], BF16, kind="ExternalOutput")
    dbg = {}
    if debug:
        dbg["F"] = nc.dram_tensor("dbgF", [SK, HW], BF16, kind="ExternalOutput")
        dbg["FR2"] = nc.dram_tensor("dbgFR2", [128, F2], BF16, kind="ExternalOutput")
        dbg["Z"] = nc.dram_tensor("dbgZ", [128, S2 * W], F32, kind="ExternalOutput")

    with tile.TileContext(nc) as tc:
        cp = _Pool(tc, name="consts", bufs=1)
        w1t_sb = cp.tile([C + 1, M], BF16)
        nc.sync.dma_start(w1t_sb[:], w1t.ap())
        wet_sb = cp.tile([M + 1, 9 * SK], BF16)
        nc.sync.dma_start(wet_sb[:], wet.ap())
        ident = cp.tile([SK, SK], BF16)
        make_identity(nc, ident[:])

        # fr2 [h, (s, k, w, 2)] outlives the whole mask pipeline
        fr2p = _Pool(tc, name="fr2", bufs=1)
        fr2 = fr2p.tile([128, F2], BF16)

        fp_ = _Pool(tc, name="fsb", bufs=1)
        f_sb = fp_.tile([SK, HW], BF16)

        rp_ = _Pool(tc, name="R", bufs=1)
        R = rp_.tile([M + 1, RP * RP], BF16)
        nc.gpsimd.memset(R[:], 0.0)
        nc.sync.dma_start(
            _ap(R, M * RP * RP, [[RP * RP, 1], [1, RP * RP]]), onesr.ap()
        )

        pin = _Pool(tc, name="xin", bufs=1)
        x_sb = pin.tile([C + 1, HW], BF16)
        nc.sync.dma_start(x_sb[:], x_aug.ap())

        # ---- conv1x1 + relu -> R (strided interior writes) ----
        psA = _Pool(tc, name="psA", bufs=3, space="PSUM")
        for n in range(32):  # 4 image rows per chunk
            ps = psA.tile([M, 512], F32, tag="ps1")
            nc.tensor.matmul(
                ps[:], w1t_sb[:], x_sb[:, n * 512 : (n + 1) * 512],
                start=True, stop=True,
            )
            nc.scalar.activation(
                _ap(R, (1 + 4 * n) * RP + 1, [[RP * RP, M], [RP, 4], [1, W]]),
                ps[:],
                mybir.ActivationFunctionType.Relu,
            )
        psA.close()
        pin.close()

        # ---- conv3x3 (9 accumulating taps) + exp -> f_sb ----
        psB = _Pool(tc, name="psB", bufs=3, space="PSUM")
        for n in range(32):
            ps = psB.tile([SK, 512], F32, tag="ps2")
            for t in range(9):
                ty, tx = divmod(t, 3)
                nc.tensor.matmul(
                    ps[:],
                    wet_sb[:, t * SK : (t + 1) * SK],
                    _ap(R, (4 * n + ty) * RP + tx, [[RP * RP, M + 1], [RP, 4], [1, W]]),
                    start=(t == 0),
                    stop=(t == 8),
                )
            nc.scalar.activation(
                f_sb[:, n * 512 : (n + 1) * 512], ps[:],
                mybir.ActivationFunctionType.Exp,
            )
        psB.close()
        rp_.close()
        if debug:
            nc.sync.dma_start(dbg["F"].ap(), f_sb[:])

        # ---- F^T via PE transposes, duplicated write into fr2 ----
        zp = _Pool(tc, name="z", bufs=1)
        zbuf = zp.tile([128, S2 * W], F32)
        psF = _Pool(tc, name="psF", bufs=4, space="PSUM")
        for w in range(W):
            pst = psF.tile([128, SK], BF16, tag="pst")
            nc.tensor.transpose(pst[:], _ap(f_sb, w, [[HW, SK], [W, H]]), ident[:])
            nc.scalar.copy(
                _ap(fr2, w * 2, [[F2, 128], [W * 2, SK], [1, 2]]),
                _ap(pst, 0, [[SK, 128], [1, SK], [0, 2]]),
            )
        psF.close()

        # ---- softmax denominator + reciprocal (from unnormalized fr2) ----
        for s in range(S2):
            nc.vector.reduce_sum(
                zbuf[:, s * W : (s + 1) * W],
                _ap(fr2, s * K2 * W * 2, [[F2, 128], [2, W], [W * 2, K2]]),
                axis=mybir.AxisListType.X,
            )
        nc.vector.reciprocal(zbuf[:], zbuf[:])
        if debug:
            nc.sync.dma_start(dbg["Z"].ap(), zbuf[:])

        # ---- normalize in place per s-group on GpSimd ----
        for s in range(S2):
            nc.gpsimd.tensor_tensor(
                _ap(fr2, s * K2 * W * 2,
                    [[F2, 128], [W * 2, K2], [2, W], [1, 2]]),
                _ap(fr2, s * K2 * W * 2,
                    [[F2, 128], [W * 2, K2], [2, W], [1, 2]]),
                _ap(zbuf, s * W,
                    [[S2 * W, 128], [0, K2], [1, W], [0, 2]]),
                op=MULT,
            )
        zp.close()
        fp_.close()
        if debug:
            nc.sync.dma_start(dbg["FR2"].ap(), fr2[:])

        # ---- 5 partition-shifted (dy) copies of the interleaved X^T ----
        xp_ = _Pool(tc, name="xt", bufs=1)
        xtd = []
        for i in range(5):  # dy = i - 2
            td = xp_.tile([128, XF], BF16, tag=f"xtd{i}")
            nc.sync.dma_start(td[:], xtq.ap()[i : i + 128, :])
            xtd.append(td)

        # ---- apply: per (s, w-half): 25 muls + in-group adds (bf16 2x),
        #      dy-partials merged in fp32 ----
        WH = W // 2  # 64 output w per half
        FH = CH * WH * 2  # 4096 free elements per half
        pp_ = _Pool(tc, name="pbuf", bufs=1)
        tp_ = _Pool(tc, name="tbuf", bufs=1)
        a32p = _Pool(tc, name="acc32", bufs=1)
        abfp = _Pool(tc, name="accbf", bufs=1)
        dims_h = [[FH, 128], [WH * 2, CH], [2, WH], [1, 2]]
        for s in range(S2):
            accbf = abfp.tile([128, CH * W * 2], BF16, tag="accbf")
            for half in range(2):
                acc32 = a32p.tile([128, FH], F32, tag="acc32")
                p = pp_.tile([128, FH], BF16, tag="p")
                for dyi in range(5):
                    for dxi in range(5):
                        k = dyi * 5 + dxi
                        in0 = _ap(
                            xtd[dyi], (half * WH + dxi) * 2,
                            [[XF, 128], [WQ * 2, CH], [2, WH], [1, 2]],
                        )
                        in1 = _ap(
                            fr2, ((s * K2 + k) * W + half * WH) * 2,
                            [[F2, 128], [0, CH], [2, WH], [1, 2]],
                        )
                        if dxi == 0:
                            nc.vector.tensor_mul(_ap(p, 0, dims_h), in0, in1)
                        else:
                            t = tp_.tile([128, FH], BF16, tag="t")
                            nc.vector.tensor_mul(_ap(t, 0, dims_h), in0, in1)
                            if dyi == 0 and dxi == 4:
                                # last add of first group lands in fp32 acc
                                nc.vector.tensor_add(acc32[:], p[:], t[:])
                            else:
                                nc.vector.tensor_add(p[:], p[:], t[:])
                    if dyi in (1, 2, 3):
                        nc.vector.tensor_add(acc32[:], acc32[:], p[:])
                    elif dyi == 4:
                        nc.vector.tensor_add(
                            _ap(accbf, half * WH * 2,
                                [[CH * W * 2, 128], [W * 2, CH], [2, WH], [1, 2]]),
                            acc32[:], p[:],
                        )
            # pixel-shuffle output: c = 2*c_hi + c_lo; c4 = c_hi//2,
            # c2 = c_hi%2, c1 = c_lo; rows 2h+c2, cols (2w+c1) contiguous
            for c2 in range(2):
                dst = bass.AP(
                    tensor=out,
                    offset=(s * 16) * (4 * HW) + c2 * (2 * W),
                    ap=[[2 * (2 * W), 128], [4 * HW, 16], [1, 2 * W]],
                )
                src = _ap(
                    accbf, c2 * (W * 2),
                    [[CH * W * 2, 128], [2 * (W * 2), 16], [1, 2 * W]],
                )
                nc.sync.dma_start(dst, src)
        abfp.close()
        a32p.close()
        tp_.close()
        pp_.close()
        xp_.close()
        fr2p.close()
        cp.close()
    nc.compile()
    return nc, dbg


def host_inputs(x_img, w_compress, b_compress, w_encoder, b_encoder):
    """Per-core input map for one image [C, H, W] (all bf16)."""
    w1t = np.concatenate(
        [w_compress[:, :, 0, 0].T, b_compress[None, :]], axis=0
    ).astype(BF_NP)
    wet = np.zeros((M + 1, 9, SK), np.float32)
    for ty in range(3):
        for tx in range(3):
            wet[:M, ty * 3 + tx, :] = w_encoder[:, :, ty, tx].T
    wet[M, 4, :] = b_encoder
    x_aug = np.ones((C + 1, HW), np.float32)
    x_aug[:C] = x_img.reshape(C, HW)
    # xtq[j, c_hi, wq, c_lo] = xpad[2*c_hi + c_lo, j-2, wq-2]
    xtq = np.zeros((WQ, CH, WQ, 2), np.float32)
    xtq[2:130, :, 2:130, :] = (
        x_img.reshape(CH, 2, H, W).transpose(2, 0, 3, 1)
    )
    return {
        "x_aug": x_aug.astype(BF_NP),
        "w1t": w1t,
        "wet": wet.reshape(M + 1, 9 * SK).astype(BF_NP),
        "xtq": xtq.reshape(WQ, XF).astype(BF_NP),
        "onesr": np.ones((1, RP * RP), BF_NP),
    }


_CACHE = {}


def kernel(x, w_compress, b_compress, w_encoder, b_encoder):
    x = np.asarray(x, np.float32)
    if "nc" not in _CACHE:
        _CACHE["nc"], _ = build_program(debug=False)
    nc = _CACHE["nc"]
    in_maps = [
        host_inputs(
            x[i],
            np.asarray(w_compress, np.float32),
            np.asarray(b_compress, np.float32),
            np.asarray(w_encoder, np.float32),
            np.asarray(b_encoder, np.float32),
        )
        for i in range(N_CORES)
    ]
    from concourse.bass_utils import run_bass_kernel_spmd

    res = run_bass_kernel_spmd(nc, in_maps, core_ids=list(range(N_CORES)))
    return np.stack(
        [res.results[i]["out"].astype(np.float32) for i in range(N_CORES)], axis=0
    )
